# revision 1
# baseline (speedup 1.0000x reference)
"""Trainium2 Bass kernel for nn_Conv2d (B=32, 256->256, 56x56, 3x3, pad=1) + bias.

Strategy
--------
Data-parallel over batch: 4 images per NeuronCore x 8 cores; weights/bias
replicated; no collectives.

Per core, the conv is computed as shifted matmuls: the input is zero-padded on
the HOST to 58-wide rows (59 rows x 58 cols per image-channel, flattened to
3422), so output position (h, w) <-> flat index h*58+w, and the 3x3 tap
(kh, kw) contribution is a matmul against the padded input shifted by the
constant offset kh*58+kw.  Each output tile [128 couts x 464 positions]
accumulates 2 (cin chunks) x 9 (taps) = 18 matmuls in one PSUM bank
(3248 = 7*464 padded output positions per image; columns w in {56,57} are
junk and stripped on the host).  Matmuls run as float32r (1 cycle/row on the
PE at N>=256; ~1.4e-4 relative error, measured on HW).  Bias is fused into
the PSUM->SBUF eviction via ScalarE activation(Identity, bias=...).
"""

import numpy as np

import concourse.bacc as bacc
import concourse.tile as tile
import concourse.mybir as mybir
from concourse.bass_utils import run_bass_kernel_spmd

F32 = mybir.dt.float32
F32R = mybir.dt.float32r

B, CIN, COUT, H, W, K = 32, 256, 256, 56, 56, 3
NCORES = 8
BPC = B // NCORES          # images per core
WP = W + 2                 # padded row width (58)
HP = H + 3                 # padded rows (59): 1 top, 2 bottom (tail tap reads)
XF = HP * WP               # padded flat length per image-channel (3422)
OF = H * WP                # padded output flat length (3248)
NT = 7                     # output tiles per (img, cout-chunk)
NFREE = OF // NT           # 464 positions per matmul (>=256 keeps f32r fast)

_CACHE = {}


def _build():
    if "nc" in _CACHE:
        return _CACHE["nc"]
    nc = bacc.Bacc("TRN2", target_bir_lowering=False, debug=False,
                   num_swdge_queues=4)
    x_d = nc.dram_tensor("x", [BPC, CIN, XF], F32R, kind="ExternalInput").ap()
    w_d = nc.dram_tensor("w", [K * K, CIN, COUT], F32R, kind="ExternalInput").ap()
    b_d = nc.dram_tensor("b", [COUT], F32, kind="ExternalInput").ap()
    o_d = nc.dram_tensor("o", [BPC, COUT, OF], F32, kind="ExternalOutput").ap()

    XLOAD = 3366  # matmuls never read past 3365

    with tile.TileContext(nc) as tc:
        with (
            tc.tile_pool(name="wp", bufs=1) as wp,
            tc.tile_pool(name="xp", bufs=6) as xp,
            tc.tile_pool(name="op", bufs=2) as op,
            tc.tile_pool(name="pp", bufs=8, space="PSUM") as pp,
        ):
            # DMA trigger instructions cost ~0.6us EACH on the issuing
            # engine, so issue in parallel from both HWDGE engines:
            # sync carries ci=0 traffic, scalar carries ci=1.
            eng = [nc.sync, nc.scalar]

            bias_t = wp.tile([128, 2], F32)
            # weights [cin-in-chunk, cin_chunk, tap, cout] in per-(cc,ci,tap)
            # 64KB DMAs: the cc=0 half (1.18MB) is all the first compute wave
            # needs; cc=1 arrives during it.
            w_t = wp.tile([128, 2, K * K, COUT], F32R)

            def w_dma(e, ci, t, cc):
                e.dma_start(
                    out=w_t[:, ci, t, cc * 128:(cc + 1) * 128],
                    in_=w_d[t, ci * 128:(ci + 1) * 128, cc * 128:(cc + 1) * 128],
                )

            def x_dma(e, xs, img, ci, lo, hi):
                e.dma_start(
                    out=xs[ci][:, lo:hi],
                    in_=x_d[img, ci * 128:(ci + 1) * 128, lo:hi],
                )

            def alloc_x():
                xs = []
                for ci in range(2):
                    x_t = xp.tile([128, XF], F32R, tag="x")
                    xs.append(x_t)
                return xs

            # slice boundaries: nt-pair p depends only on x up to
            # 582+464*(2p+1), so early pairs unblock as slices land
            xsl = [0, 291, 582, 1046, 1510, 1974, 2438, 2902, XLOAD]

            def load_img(img):
                xs = alloc_x()
                for s in range(len(xsl) - 1):
                    for ci in range(2):
                        x_dma(eng[ci], xs, img, ci, xsl[s], xsl[s + 1])
                return xs

            def load_img0():
                # Hand-scheduled startup: DMA triggers cost ~0.65us each on
                # the issuing engine; sync (ci=0) and scalar (ci=1) carry
                # first-wave weights + x interleaved by consumption time
                xs = alloc_x()
                for ci in range(2):
                    e = eng[ci]
                    w_dma(e, ci, 0, 0)
                    x_dma(e, xs, 0, ci, xsl[0], xsl[1])
                    x_dma(e, xs, 0, ci, xsl[1], xsl[2])
                    x_dma(e, xs, 0, ci, xsl[2], xsl[3])
                    w_dma(e, ci, 1, 0)
                    w_dma(e, ci, 2, 0)
                    x_dma(e, xs, 0, ci, xsl[3], xsl[4])
                    w_dma(e, ci, 3, 0)
                    w_dma(e, ci, 4, 0)
                    x_dma(e, xs, 0, ci, xsl[4], xsl[5])
                    w_dma(e, ci, 5, 0)
                    w_dma(e, ci, 6, 0)
                    x_dma(e, xs, 0, ci, xsl[5], xsl[6])
                    w_dma(e, ci, 7, 0)
                    w_dma(e, ci, 8, 0)
                    e.dma_start(out=bias_t[:, ci:ci + 1],
                                in_=b_d[ci * 128:(ci + 1) * 128])
                    x_dma(e, xs, 0, ci, xsl[6], xsl[7])
                    x_dma(e, xs, 0, ci, xsl[7], xsl[8])
                return xs

            def do_group(xs, cc, o_t, img, nts, fine_stores=False):
                """One PSUM accumulation wave over nt tiles `nts` (1 or 2),
                sharing each weight tile across the wave to halve LDWEIGHTS
                pressure on the PE."""
                pss = []
                for nt in nts:
                    ps = pp.tile([128, NFREE], F32, tag="ps")
                    pss.append(ps)
                for mi, (ci, t) in enumerate(
                    [(ci, t) for ci in range(2) for t in range(K * K)]
                ):
                    kh, kw = divmod(t, K)
                    for ps, nt in zip(pss, nts):
                        off = nt * NFREE + kh * WP + kw
                        nc.tensor.matmul(
                            ps,
                            w_t[:, ci, t, cc * 128:(cc + 1) * 128],
                            xs[ci][:, off:off + NFREE],
                            start=(mi == 0),
                            stop=(mi == 17),
                        )
                for ps, nt in zip(pss, nts):
                    # bias-add + PSUM eviction on the otherwise-idle DVE
                    nc.vector.tensor_scalar_add(
                        o_t[:, nt * NFREE:(nt + 1) * NFREE],
                        ps,
                        bias_t[:, cc:cc + 1],
                    )
                    # store each nt slice as soon as its bias-add finishes,
                    # halves split across the issue engines (quarters for
                    # the final group so the drain tail stays short)
                    nsp = 4 if fine_stores else 2
                    q = NFREE // nsp
                    for s in range(nsp):
                        h0 = nt * NFREE + s * q
                        eng[s % 2].dma_start(
                            out=o_d[img, cc * 128:(cc + 1) * 128, h0:h0 + q],
                            in_=o_t[:, h0:h0 + q],
                        )

            for img in range(BPC):
                if img == 0:
                    xs = load_img0()
                    # cc=1 weights via SWDGE, needed ~27us in
                    for ci in range(2):
                        for t in range(K * K):
                            w_dma(nc.gpsimd, ci, t, 1)
                else:
                    xs = load_img(img)
                for cc in range(2):
                    o_t = op.tile([128, OF], F32, tag="o")
                    last = img == BPC - 1 and cc == 1
                    for nts in ([0, 1], [2, 3], [4, 5], [6]):
                        do_group(xs, cc, o_t, img, nts,
                                 fine_stores=last and nts == [6])
    nc.compile()
    _CACHE["nc"] = nc
    return nc


def make_in_maps(inp, kernel, bias):
    xpad = np.zeros((B, CIN, HP, WP), np.float32)
    xpad[:, :, 1:1 + H, 1:1 + W] = inp
    xflat = xpad.reshape(B, CIN, XF)
    # [cout, cin, kh, kw] -> [tap(kh*3+kw), cin, cout]
    w_dev = np.ascontiguousarray(
        np.asarray(kernel, np.float32).transpose(2, 3, 1, 0).reshape(K * K, CIN, COUT)
    )
    b_dev = np.ascontiguousarray(np.asarray(bias, np.float32))
    return [
        {"x": np.ascontiguousarray(xflat[c * BPC:(c + 1) * BPC]),
         "w": w_dev, "b": b_dev}
        for c in range(NCORES)
    ]


def assemble(results):
    o = np.concatenate([results[c]["o"] for c in range(NCORES)], axis=0)
    return np.ascontiguousarray(
        o.reshape(B, COUT, H, WP)[:, :, :, :W].astype(np.float32)
    )


def kernel(inp, kernel, bias):
    nc = _build()
    in_maps = make_in_maps(inp, kernel, bias)
    r = run_bass_kernel_spmd(nc, in_maps, core_ids=list(range(NCORES)))
    return assemble(r.results)



# revision 4
# speedup vs baseline: 1.4135x; 1.4135x over previous
"""Trainium2 Bass kernel for nn_Conv2d (B=32, 256->256, 56x56, 3x3, pad=1) + bias.

Strategy
--------
Data-parallel over batch: 4 images per NeuronCore x 8 cores; weights/bias
replicated; no collectives.

Per core, the conv uses 1-D Winograd F(2,3) along W: each pair of output
columns (2tx, 2tx+1) is produced from 4 "points" p, where
M_p[h,tx] = sum_{kh,cin} Wt[p,kh,cin,cout] * V_p[cin, h+kh, tx] and
  out[h,2tx]   = M0 + M1 + M2 + bias
  out[h,2tx+1] = M1 - M2 - M3 + bias
This cuts PE rows streamed 1.5x vs direct shifted-matmul (4 point-streams of
28 tiles vs 3 kw-taps of 56+2 columns).  The input transform V (4 add/subs
over even/odd column phases, host-pre-split for unit stride) runs on the
otherwise-idle Vector engine, prefetched one image ahead; the output
transform + bias is fused into PSUM eviction via scalar_tensor_tensor.
Weights are host-transformed (G @ w along kw, exact halves) and everything
the PE touches is fp16 (1 cycle/row like f32r, but enables fast weight load
and halves DMA); accumulation is f32 in PSUM.  Measured rel err ~3.6e-4.
"""

import numpy as np

import concourse.bacc as bacc
import concourse.tile as tile
import concourse.mybir as mybir
from concourse.bass_utils import run_bass_kernel_spmd

F32 = mybir.dt.float32
F16 = mybir.dt.float16
ALU = mybir.AluOpType

B, CIN, COUT, H, W, K = 32, 256, 256, 56, 56, 3
NCORES = 8
BPC = B // NCORES          # images per core
HP = H + 2                 # padded rows (1 top, 1 bottom)
NT = 28                    # winograd tiles along W (2 output cols each)
XF = HP * 2 * 29           # x elems per (img, cinc): rows x (even|odd) x 29
SROWS = 14                 # output rows per PSUM chunk
NFREE = SROWS * NT         # 392 free positions per matmul
NS = H // SROWS            # 4 chunks per (img, cc)

_CACHE = {}


def _build():
    if "nc" in _CACHE:
        return _CACHE["nc"]
    nc = bacc.Bacc("TRN2", target_bir_lowering=False, debug=False,
                   num_swdge_queues=4)
    x_d = nc.dram_tensor("x", [BPC, 2, 128, XF], F16, kind="ExternalInput").ap()
    w_d = nc.dram_tensor("w", [4, K, 2, 128, COUT], F16, kind="ExternalInput").ap()
    b_d = nc.dram_tensor("b", [COUT], F32, kind="ExternalInput").ap()
    # device layout [img, cout, h, par, tx]; host interleaves par/tx -> w
    o_d = nc.dram_tensor("o", [BPC, COUT, H * W], F32, kind="ExternalOutput").ap()

    with tile.TileContext(nc) as tc:
        with (
            tc.tile_pool(name="wp", bufs=1) as wp,
            tc.tile_pool(name="xp", bufs=4) as xp,
            tc.tile_pool(name="vp", bufs=4) as vp,
            tc.tile_pool(name="op", bufs=3) as op,
            tc.tile_pool(name="sp", bufs=6) as sp,
            tc.tile_pool(name="pp", bufs=8, space="PSUM") as pp,
        ):
            eng = [nc.sync, nc.scalar]

            bias_t = wp.tile([128, 2], F32)
            w_t = wp.tile([128, 4, K, 2, COUT], F16)

            # weights + bias ride the gpsimd SWDGE queues (keeps the sync/
            # scalar HWDGE queues free for x/out); cc=0 halves first so the
            # first matmul group can start early.
            for cc in range(2):
                for p in range(4):
                    for kh in range(K):
                        for ci in range(2):
                            nc.gpsimd.dma_start(
                                out=w_t[:, p, kh, ci, cc * 128:(cc + 1) * 128],
                                in_=w_d[p, kh, ci, :, cc * 128:(cc + 1) * 128],
                            )
                nc.gpsimd.dma_start(out=bias_t[:, cc:cc + 1],
                                    in_=b_d[cc * 128:(cc + 1) * 128])

            # x row-chunks: chunk A (rows 0..29) unlocks s=0,1; B the rest
            RC = [(0, 30), (30, HP)]

            def load_x(img):
                xs = []
                for ci in range(2):
                    x_t = xp.tile([128, HP, 2, 29], F16, tag="x")
                    for r0, r1 in RC:
                        eng[ci].dma_start(
                            out=x_t[:, r0:r1],
                            in_=x_d[img, ci, :, r0 * 58:r1 * 58],
                        )
                    xs.append(x_t)
                return xs

            def v_transform(xs):
                vs = []
                for ci in range(2):
                    v_t = vp.tile([128, 4, HP, NT], F16, tag="v")
                    vs.append(v_t)
                for r0, r1 in RC:
                    for ci in range(2):
                        ev0 = xs[ci][:, r0:r1, 0, 0:28]
                        ev1 = xs[ci][:, r0:r1, 0, 1:29]
                        od0 = xs[ci][:, r0:r1, 1, 0:28]
                        od1 = xs[ci][:, r0:r1, 1, 1:29]
                        v = vs[ci]
                        nc.vector.tensor_sub(v[:, 0, r0:r1], ev0, ev1)
                        nc.vector.tensor_add(v[:, 1, r0:r1], od0, ev1)
                        nc.vector.tensor_sub(v[:, 2, r0:r1], ev1, od0)
                        nc.vector.tensor_sub(v[:, 3, r0:r1], od0, od1)
                return vs

            def do_group(vs, cc, s, o_t):
                ms = []
                for p in range(4):
                    m = pp.tile([128, NFREE], F32, tag="ps")
                    for mi, (kh, ci) in enumerate(
                        [(kh, ci) for kh in range(K) for ci in range(2)]
                    ):
                        nc.tensor.matmul(
                            m,
                            w_t[:, p, kh, ci, cc * 128:(cc + 1) * 128],
                            vs[ci][:, p, SROWS * s + kh:SROWS * s + kh + SROWS],
                            start=(mi == 0),
                            stop=(mi == 5),
                        )
                    ms.append(m)
                # DVE ops may read at most ONE PSUM input each, so M2 is
                # evicted by the otherwise-idle ACT engine first.
                bias_ap = bias_t[:, cc:cc + 1]
                tm2 = sp.tile([128, NFREE], F32, tag="t")
                sa = sp.tile([128, NFREE], F32, tag="t")
                sb = sp.tile([128, NFREE], F32, tag="t")
                nc.scalar.copy(tm2, ms[2])
                nc.vector.scalar_tensor_tensor(          # M1 + b + M2
                    sa, ms[1], bias_ap, tm2, ALU.add, ALU.add)
                nc.vector.tensor_add(                    # out0 = M0 + sa
                    o_t[:, SROWS * s:SROWS * (s + 1), 0], ms[0], sa)
                nc.vector.scalar_tensor_tensor(          # sa - 2*M2 = M1-M2+b
                    sb, tm2, -2.0, sa, ALU.mult, ALU.add)
                nc.vector.tensor_sub(                    # out1 = sb - M3
                    o_t[:, SROWS * s:SROWS * (s + 1), 1], sb, ms[3])

            xs_all = [load_x(0), load_x(1)]
            vs_cur = None
            vs_next = v_transform(xs_all[0])
            for img in range(BPC):
                vs_cur = vs_next
                if img + 1 < BPC:
                    if img + 2 < BPC:
                        xs_all.append(load_x(img + 2))
                    vs_next = v_transform(xs_all[img + 1])
                for cc in range(2):
                    o_t = op.tile([128, H, 2, NT], F32, tag="o")
                    for s in range(NS):
                        do_group(vs_cur, cc, s, o_t)
                    # store halves split across the two issue engines
                    hh = H // 2
                    for si in range(2):
                        eng[si].dma_start(
                            out=o_d[img, cc * 128:(cc + 1) * 128,
                                    si * hh * W:(si + 1) * hh * W],
                            in_=o_t[:, si * hh:(si + 1) * hh],
                        )
    nc.compile()
    _CACHE["nc"] = nc
    return nc


def make_in_maps(inp, kernel, bias):
    xpad = np.zeros((B, CIN, HP, W + 2), np.float32)
    xpad[:, :, 1:1 + H, 1:1 + W] = inp
    ev = xpad[:, :, :, 0::2]
    od = xpad[:, :, :, 1::2]
    x_par = np.stack([ev, od], axis=3).astype(np.float16)   # [B,CIN,58,2,29]
    x_dev = np.ascontiguousarray(
        x_par.reshape(B, 2, 128, XF))

    w = np.asarray(kernel, np.float64)                      # [cout,cin,kh,kw]
    G = np.array([[1, 0, 0], [.5, .5, .5], [.5, -.5, .5], [0, 0, 1]], np.float64)
    Wt = np.einsum("pk,ochk->poch", G, w)                   # [4,cout,cin,kh]
    w_dev = np.ascontiguousarray(
        Wt.transpose(0, 3, 2, 1).reshape(4, K, 2, 128, COUT).astype(np.float16))
    b_dev = np.ascontiguousarray(np.asarray(bias, np.float32))
    return [
        {"x": np.ascontiguousarray(x_dev[c * BPC:(c + 1) * BPC]),
         "w": w_dev, "b": b_dev}
        for c in range(NCORES)
    ]


def assemble(results):
    o = np.concatenate([results[c]["o"] for c in range(NCORES)], axis=0)
    # device layout [.., h, par, tx] -> [.., h, 2tx+par]
    o = o.reshape(B, COUT, H, 2, NT).transpose(0, 1, 2, 4, 3)
    return np.ascontiguousarray(o.reshape(B, COUT, H, W).astype(np.float32))


def kernel(inp, kernel, bias):
    nc = _build()
    in_maps = make_in_maps(inp, kernel, bias)
    r = run_bass_kernel_spmd(nc, in_maps, core_ids=list(range(NCORES)))
    return assemble(r.results)
